# revision 24
# baseline (speedup 1.0000x reference)
"""GCN + 2-step APPNP propagation on 8 Trainium2 NeuronCores.

Reference computation (N=16384, NFEAT=500, HIDDEN=32, NCLASS=3, alpha=0.25):
    h   = relu(input @ W1)
    l0  = h @ W2
    deg = adj.sum(axis=1);  d = (1 - alpha) / max(deg, 1e-12)
    l1  = d * (adj @ l0) + alpha * l0
    l2  = d * (adj @ l1) + alpha * l0
    out = log_softmax(l2, axis=1)

Distribution: 1D row partition; core r owns rows r*2048..(r+1)*2048 and
streams T_r = adj[rows_r, :].T as fp8-e4m3 in 16 blocks of [128, 16384]
(2 MiB, 16 KiB per-partition lines).

Key structure vs the naive two-full-sweeps version:
 - The first NRES=8 blocks are RESIDENT: loaded once in pass 1 into
   persistent SBUF tiles and reused by pass 2, cutting HBM traffic from
   ~68 MiB to ~50 MiB per core.  Only blocks 8..15 are re-streamed.
 - AllGather payloads are compact [P, 16, 4] fp8 (8 KiB in / 64 KiB out)
   instead of LPAD-padded 32 KiB; the gathered logits are expanded to the
   stride-16 DoubleRow stationary layout by a cheap on-chip copy.
 - Stage 1 runs as soon as xt lands (one 2 MiB padded-layout DMA), so the
   l0 AllGather triggers at ~25 us; the collective engine's one-time
   startup (~50 us, schedule-invariant on this runtime) then bounds the
   pass-1 matmul start at ~95 us, with ~12 blocks prefetched meanwhile.
 - The bulk adj stream rides the scalar-engine hardware DMA queue; the
   collective bounce/gather DMAs ride the otherwise-empty sync queue.
   Keeping extra blocked waiters off the sync queue matters: every
   attempt to split or pin DMAs there slowed the collectives themselves.
 - Both passes consume blocks in an interleaved resident/streamed order
   so stream-buffer releases stay ahead of the PE; each pass runs as one
   contiguous ~57 us PE-bound window (fp8 DoubleRow ingestion floor).

Both propagation passes use fp8 DoubleRow matmuls: stationary = logits
chunk-pair [128, 2, 4-of-16], moving = streamed/resident adj block slice
[128, 2, 512], accumulating over all 128 column chunks into [4, 512]
PSUM slices.  deg rides pass 1 as a ones-column.  Output leaves
chunk-major [128, 16, 3] and is un-permuted on the host.
"""

import os

import numpy as np
import ml_dtypes

import concourse.bass as bass
import concourse.mybir as mybir
import concourse.bacc as bacc
import concourse.tile as tile
from concourse import bass_utils
from concourse.bass import _add_dep_helper

N = 16384
NFEAT = 500
HIDDEN = 32
NCLASS = 3
ALPHA = 0.25
NCORES = 8
ROWS = N // NCORES        # 2048 rows owned per core
P = 128                   # SBUF partitions
CHUNKS = N // P           # 128 global column-chunks
LCH = ROWS // P           # 16 local row-chunks
NB = 8                    # column-chunks per adj DMA block
NBLK = N // (NB * P)      # 16 blocks per pass
ISL = 512                 # moving-operand free-dim per matmul
NISL = ROWS // ISL        # 4 output column slices
NRES = 8                  # blocks resident in SBUF across both passes
TT_BUFS = 3               # stream-pool depth for the re-streamed blocks
LPAD = 16                 # stationary chunk stride (DoubleRow: step%16==0)
KF = 512                  # padded feature rows (500 -> 4*128)
# PE consumption orders.  Pass 1 trails the stream, so natural order is
# right.  Pass 2 starts from SBUF-resident blocks interleaved with the
# re-streamed ones so buffer releases stay ahead of the PE.
ORDER1 = [0, 8, 1, 9, 2, 10, 3, 11, 4, 12, 5, 13, 6, 14, 7, 15]
ORDER2 = [0, 8, 1, 9, 2, 10, 3, 11, 4, 12, 5, 13, 6, 14, 7, 15]

F32 = mybir.dt.float32
BF16 = mybir.dt.bfloat16
ADT = mybir.dt.float8e4
ADT_NP = ml_dtypes.float8_e4m3
BF16_NP = ml_dtypes.bfloat16
AF = mybir.ActivationFunctionType
ALU = mybir.AluOpType
AX = mybir.AxisListType
DR = mybir.MatmulPerfMode.DoubleRow

_COMPILED = None
LAST_EXEC_TIME_NS = None
LAST_RESULTS = None


def _build():
    nc = bacc.Bacc("TRN2", target_bir_lowering=False, debug=False,
                   num_devices=NCORES)

    t_d = nc.dram_tensor("t", [NBLK, P, NB * ROWS], ADT,
                         kind="ExternalInput").ap()
    xt_d = nc.dram_tensor("xt", [P, 4 * ROWS], BF16, kind="ExternalInput").ap()
    w1_d = nc.dram_tensor("w1", [P, 4 * HIDDEN], BF16,
                          kind="ExternalInput").ap()
    w2_d = nc.dram_tensor("w2", [HIDDEN, NCLASS], BF16,
                          kind="ExternalInput").ap()
    eye_d = nc.dram_tensor("eye", [4, 4], F32, kind="ExternalInput").ap()
    out_d = nc.dram_tensor("out", [P, LCH * NCLASS], F32,
                           kind="ExternalOutput").ap()

    rg = [list(range(NCORES))]

    with tile.TileContext(nc) as tc:
        with (
            tc.tile_pool(name="const", bufs=1) as const,
            tc.tile_pool(name="persist", bufs=1) as persist,
            tc.tile_pool(name="res", bufs=1) as res,
            tc.tile_pool(name="ttp", bufs=TT_BUFS) as ttp,
            tc.tile_pool(name="dram", bufs=1, space="DRAM") as dram,
        ):
            eye_sb = const.tile([4, 4], F32)
            nc.gpsimd.dma_start(eye_sb[:], eye_d[:])
            w2_sb = const.tile([HIDDEN, NCLASS], BF16)
            nc.gpsimd.dma_start(w2_sb[:], w2_d[:])
            w1_sb = const.tile([P, 4, HIDDEN], BF16)
            nc.gpsimd.dma_start(w1_sb[:],
                                w1_d[:].rearrange("p (k h) -> p k h", k=4))

            # live across the whole kernel
            alpha_l0 = persist.tile([P, LCH, NCLASS], F32)    # 0.25*l0, local
            d_all = persist.tile([P, LCH], F32)               # 0.75/deg, local
            l0_rhs = persist.tile([P, CHUNKS, LPAD], ADT)     # [l0 | 1] chunks
            l1_rhs = l0_rhs                                   # pass-2 reuses
            l0c = persist.tile([P, LCH, 4], ADT)              # local AG payload
            l1c = persist.tile([P, LCH, 4], ADT)              # local AG payload
            l0g = persist.tile([P, NCORES, LCH * 4], ADT)     # gathered compact
            l1g = l0g
            out_sb = persist.tile([P, LCH, NCLASS], F32)
            y1T = persist.tile([4, ROWS], F32)     # pass-2 reuses rows 0:3
            y2T = y1T
            xt_sb = persist.tile([P, 4, ROWS], BF16)          # padded features

            res_t = [res.tile([P, NB * ROWS], ADT, name=f"res{b}")
                     for b in range(NRES)]

            cc1_in = dram.tile([ROWS * 4], ADT)
            cc2_in = dram.tile([ROWS * 4], ADT)
            cc1_out = nc.dram_tensor("cc1_out", [N * 4], ADT,
                                     kind="Internal",
                                     addr_space="Shared").ap()
            cc2_out = nc.dram_tensor("cc2_out", [N * 4], ADT,
                                     kind="Internal",
                                     addr_space="Shared").ap()

            def stream_block(idx):
                # bulk stream on the scalar-engine hardware DMA queue: the
                # collectives' transfers drain behind the sync ring's
                # backlog, so the sync ring must stay shallow
                tt = ttp.tile([P, NB * ROWS], ADT, name="tt", tag="tt")
                return tt, nc.scalar.dma_start(tt[:], t_d[idx])

            # ---- head of the sync FIFO: xt, then resident blocks ----------
            xt_dma = nc.scalar.dma_start(
                xt_sb[:], xt_d[:].rearrange("p (k f) -> p k f", k=4))
            res_dma = []

            # ---- stage 1: l0 = relu(x @ W1) @ W2 (transposed forms) -------
            with (
                tc.tile_pool(name="s1sb", bufs=1) as s1sb,
                tc.tile_pool(name="hpsp", bufs=1, space="PSUM") as hpsp,
                tc.tile_pool(name="l0psp", bufs=1, space="PSUM") as l0psp,
            ):
                hps = [hpsp.tile([HIDDEN, ISL], F32, name=f"hps{i}",
                                 tag=f"hps{i}") for i in range(NISL)]
                for k in range(4):
                    for i in range(NISL):
                        nc.tensor.matmul(
                            hps[i][:], w1_sb[:, k, :],
                            xt_sb[:, k, i * ISL:(i + 1) * ISL],
                            start=(k == 0), stop=(k == 3))
                hT = s1sb.tile([HIDDEN, ROWS], BF16)
                for i in range(NISL):
                    nc.scalar.activation(hT[:, i * ISL:(i + 1) * ISL],
                                         hps[i][:], AF.Relu)

                l0ps = l0psp.tile([P, LCH, NCLASS], F32)
                for n in range(LCH):
                    nc.tensor.matmul(l0ps[:, n, :], hT[:, n * P:(n + 1) * P],
                                     w2_sb[:], start=True, stop=True)
                nc.vector.tensor_scalar_mul(alpha_l0[:], l0ps[:], ALPHA)
                nc.scalar.activation(l0c[:, :, 0:NCLASS], l0ps[:], AF.Copy)
                nc.vector.memset(l0c[:, :, 3], 1.0)
                nc.vector.memset(l1c[:, :, 3], 0.0)

            # ---- all-gather l0 (compact fp8, own queue off the stream) ----
            cc1_w = nc.sync.dma_start(
                cc1_in[:].rearrange("(p f) -> p f", p=P),
                l0c[:].rearrange("p n f -> p (n f)"))
            ag1 = nc.gpsimd.collective_compute(
                "AllGather", ALU.bypass, replica_groups=rg,
                ins=[cc1_in.opt()], outs=[cc1_out.opt()])
            g1 = nc.sync.dma_start(
                l0g[:],
                cc1_out[:].rearrange("(k p f) -> p k f", k=NCORES, p=P))
            # expand compact [P, 128, 4] to the stride-16 stationary layout
            nc.vector.tensor_copy(
                l0_rhs[:].rearrange("p (k n) f -> p k n f", k=NCORES)
                [:, :, :, 0:4],
                l0g[:].rearrange("p k (n f) -> p k n f", f=4))

            for b in range(NRES):
                res_dma.append(nc.scalar.dma_start(res_t[b][:], t_d[b]))
            p1 = [stream_block(b) for b in range(NRES, NBLK)]

            # ---- propagation pass 1: y1 = adj @ [l0 | 1] ------------------
            with tc.tile_pool(name="y1ps", bufs=1, space="PSUM") as y1psp:
                y1ps = [y1psp.tile([4, ISL], F32, name=f"y1ps{i}",
                                   tag=f"y1ps{i}") for i in range(NISL)]
                for bi, b in enumerate(ORDER1):
                    src = res_t[b] if b < NRES else p1[b - NRES][0]
                    tt3 = src[:].rearrange("p (s f) -> p s f", s=NB)
                    for s2 in range(NB // 2):
                        jc = b * NB + 2 * s2
                        nfirst = bi == 0 and s2 == 0
                        nlast = bi == NBLK - 1 and s2 == NB // 2 - 1
                        for i in range(NISL):
                            nc.tensor.matmul(
                                y1ps[i][:], l0_rhs[:, jc:jc + 2, 0:4],
                                tt3[:, 2 * s2:2 * s2 + 2,
                                    i * ISL:(i + 1) * ISL],
                                start=nfirst, stop=nlast,
                                perf_mode=DR)
                for i in range(NISL):
                    nc.scalar.activation(y1T[:, i * ISL:(i + 1) * ISL],
                                         y1ps[i][:], AF.Copy)

            # ---- iteration update: l1 = d*y1 + alpha*l0 -------------------
            with (
                tc.tile_pool(name="upd", bufs=1) as upd,
                tc.tile_pool(name="updps", bufs=1, space="PSUM") as updps,
            ):
                ytp = updps.tile([P, LCH, 4], F32)
                for n in range(LCH):
                    nc.tensor.transpose(ytp[:, n, :],
                                        y1T[:, n * P:(n + 1) * P], eye_sb[:])
                dmx = upd.tile([P, LCH], F32)
                nc.vector.tensor_scalar_max(dmx[:], ytp[:, :, 3], 1e-12)
                rec = upd.tile([P, LCH], F32)
                nc.vector.reciprocal(rec[:], dmx[:])
                nc.vector.tensor_scalar_mul(d_all[:], rec[:], 1.0 - ALPHA)
                ty = upd.tile([P, LCH, NCLASS], F32)
                nc.vector.tensor_mul(ty[:], ytp[:, :, 0:NCLASS],
                                     d_all[:].broadcast_to([P, LCH, NCLASS]))
                tyf = upd.tile([P, LCH, NCLASS], F32)
                nc.vector.tensor_add(tyf[:], ty[:], alpha_l0[:])
                nc.scalar.activation(l1c[:, :, 0:NCLASS], tyf[:], AF.Copy)

            # ---- pass-2 stream + all-gather l1 ----------------------------
            p2 = [stream_block(NRES)]
            cc2_w = nc.sync.dma_start(
                cc2_in[:].rearrange("(p f) -> p f", p=P),
                l1c[:].rearrange("p n f -> p (n f)"))
            ag2 = nc.gpsimd.collective_compute(
                "AllGather", ALU.bypass, replica_groups=rg,
                ins=[cc2_in.opt()], outs=[cc2_out.opt()])
            g2 = nc.sync.dma_start(
                l1g[:],
                cc2_out[:].rearrange("(k p f) -> p k f", k=NCORES, p=P))
            nc.vector.tensor_copy(
                l1_rhs[:].rearrange("p (k n) f -> p k n f", k=NCORES)
                [:, :, :, 0:4],
                l1g[:].rearrange("p k (n f) -> p k n f", f=4))
            for b in range(NRES + 1, NBLK):
                p2.append(stream_block(b))

            # ---- propagation pass 2: y2 = adj @ l1 ------------------------
            with tc.tile_pool(name="y2ps", bufs=1, space="PSUM") as y2psp:
                y2ps = [y2psp.tile([NCLASS, ISL], F32, name=f"y2ps{i}",
                                   tag=f"y2ps{i}") for i in range(NISL)]
                for bi, b in enumerate(ORDER2):
                    src = res_t[b] if b < NRES else p2[b - NRES][0]
                    tt3 = src[:].rearrange("p (s f) -> p s f", s=NB)
                    for s2 in range(NB // 2):
                        jc = b * NB + 2 * s2
                        nfirst = bi == 0 and s2 == 0
                        nlast = bi == NBLK - 1 and s2 == NB // 2 - 1
                        for i in range(NISL):
                            nc.tensor.matmul(
                                y2ps[i][:], l1_rhs[:, jc:jc + 2, 0:NCLASS],
                                tt3[:, 2 * s2:2 * s2 + 2,
                                    i * ISL:(i + 1) * ISL],
                                start=nfirst, stop=nlast,
                                perf_mode=DR)
                for i in range(NISL):
                    nc.scalar.activation(y2T[0:NCLASS, i * ISL:(i + 1) * ISL],
                                         y2ps[i][:], AF.Copy)

            # ---- final update + log_softmax -------------------------------
            with (
                tc.tile_pool(name="fin", bufs=1) as fin,
                tc.tile_pool(name="finps", bufs=1, space="PSUM") as finps,
            ):
                y2tp = finps.tile([P, LCH, NCLASS], F32)
                for n in range(LCH):
                    nc.tensor.transpose(y2tp[:, n, :],
                                        y2T[0:NCLASS, n * P:(n + 1) * P],
                                        eye_sb[0:NCLASS, 0:NCLASS])
                lg = fin.tile([P, LCH, NCLASS], F32)
                nc.vector.tensor_mul(lg[:], y2tp[:],
                                     d_all[:].broadcast_to([P, LCH, NCLASS]))
                nc.vector.tensor_add(lg[:], lg[:], alpha_l0[:])
                negm = fin.tile([P, LCH], F32)
                nc.vector.tensor_reduce(negm[:], lg[:], axis=AX.X, op=ALU.max,
                                        negate=True)
                lgm = fin.tile([P, LCH, NCLASS], F32)
                nc.vector.tensor_add(lgm[:], lg[:],
                                     negm[:].broadcast_to([P, LCH, NCLASS]))
                ex = fin.tile([P, LCH, NCLASS], F32)
                nc.scalar.activation(ex[:], lgm[:], AF.Exp)
                sm = fin.tile([P, LCH], F32)
                nc.vector.tensor_reduce(sm[:], ex[:], axis=AX.X, op=ALU.add)
                rs = fin.tile([P, LCH], F32)
                nc.vector.reciprocal(rs[:], sm[:])
                nls = fin.tile([P, LCH], F32)
                nc.scalar.activation(nls[:], rs[:], AF.Ln)
                nc.vector.tensor_add(out_sb[:], lgm[:],
                                     nls[:].broadcast_to([P, LCH, NCLASS]))

            nc.sync.dma_start(out_d[:],
                               out_sb[:].rearrange("p n f -> p (n f)"))

    nc.compile()
    return nc


def kernel(input, adj, W1, W2):
    """Full inputs in, full [N, NCLASS] float32 log-softmax out."""
    global _COMPILED, LAST_EXEC_TIME_NS, LAST_RESULTS
    if _COMPILED is None:
        _COMPILED = _build()
    nc = _COMPILED

    input = np.asarray(input, dtype=np.float32)
    adj = np.asarray(adj, dtype=np.float32)
    W1 = np.asarray(W1, dtype=np.float32)
    W2 = np.asarray(W2, dtype=np.float32)

    adj_q = adj.astype(ADT_NP)
    w1_pad = np.zeros((KF, HIDDEN), dtype=np.float32)
    w1_pad[:NFEAT] = W1
    w1_perm = np.ascontiguousarray(
        w1_pad.reshape(4, P, HIDDEN).transpose(1, 0, 2)
        .reshape(P, 4 * HIDDEN)).astype(BF16_NP)
    eye = np.eye(4, dtype=np.float32)

    in_maps = []
    for r in range(NCORES):
        t_r = np.ascontiguousarray(
            adj_q[r * ROWS:(r + 1) * ROWS, :].T
            .reshape(NBLK, NB, P, ROWS)
            .transpose(0, 2, 1, 3)
            .reshape(NBLK, P, NB * ROWS))
        xt_pad = np.zeros((KF, ROWS), dtype=np.float32)
        xt_pad[:NFEAT] = input[r * ROWS:(r + 1) * ROWS, :].T
        xt_perm = np.ascontiguousarray(
            xt_pad.reshape(4, P, ROWS).transpose(1, 0, 2)
            .reshape(P, 4 * ROWS)).astype(BF16_NP)
        in_maps.append({
            "t": t_r,
            "xt": xt_perm,
            "w1": w1_perm,
            "w2": W2.astype(BF16_NP),
            "eye": eye,
        })

    res = bass_utils.run_bass_kernel_spmd(
        nc, in_maps, core_ids=list(range(NCORES)),
        trace=bool(os.environ.get("GNN_TRACE")))
    LAST_EXEC_TIME_NS = res.exec_time_ns
    LAST_RESULTS = res

    out = np.empty((N, NCLASS), dtype=np.float32)
    for r in range(NCORES):
        blk = res.results[r]["out"].reshape(P, LCH, NCLASS)
        out[r * ROWS:(r + 1) * ROWS] = (
            blk.transpose(1, 0, 2).reshape(ROWS, NCLASS))
    return out
